# revision 9
# baseline (speedup 1.0000x reference)
"""Multi-head causal self-attention (B=8, S=1024, D=768, H=12) on 8 TRN2
NeuronCores, data-parallel over the batch (one batch element per core).

Per-core pipeline, engine-balanced (PE / ACT / DVE each ~65% busy):

  - fp16 datapath (same ~11-bit effective mantissa as the fp32r PE mode)
    with fp8-e4m3 DoubleRow matmuls where precision allows:
      * q/k projection: x and Wqk quantized to fp8 (scales 4 / 64, host-
        side for W), contraction pairs of 128-row tiles packed into the
        DoubleRow sub-row dim -> 4x MAC throughput vs fp32r
      * scores: q,k re-quantized to fp8 at scale 16 on evacuation; the
        64-dim head contraction zero-pads the second DoubleRow sub-row
        (stream cost is per output column, so padding is free) -> 2x
      * v / PV / output projection stay fp16 (fp8 would not meet the
        accuracy budget; scale factors fold into the exp activation)
  - xT built by PE bootstrap transposes (x rows 0:512, overlapping the
    input DMA stream) and DMA xbar transposes (rows 512:1024)
  - attention per head pair over q-chunks: scoresT[kp,qp] -> exp on ACT
    (the throughput-limiting engine; scale folded) -> causal tri-mask on
    the diagonal blocks (DVE, fp16 2x) -> PV with a ones column yielding
    the softmax denominator -> reciprocal + gpsimd partition_broadcast +
    multiply into attnT
  - a software-pipelined schedule runs pair j's chunk-1 attention after
    pair (j+2)'s chunk-0, spreading exp load evenly, and weaves qkT / v /
    output-projection blocks between attention steps sized to cover ACT
    latency; the last pair runs in 256-column halves so the final output
    tiles start as soon as the first half is normalized
"""

import sys

import numpy as np

for _p in ("/opt/trn_rl_repo", "/root/.axon_site/_ro/trn_rl_repo"):
    if _p not in sys.path:
        sys.path.append(_p)

import concourse.mybir as mybir  # noqa: E402
import concourse.tile as tile  # noqa: E402
from concourse import bacc  # noqa: E402
from concourse.bass_utils import run_bass_kernel_spmd  # noqa: E402

F32 = mybir.dt.float32
F16 = mybir.dt.float16
F8 = mybir.dt.float8e4

B, S, D = 8, 1024, 768
H, HD = 12, 64
ND3 = 3 * D
SCALE = 0.125  # 1/sqrt(64)
Q8 = 16.0         # fp8 quantization scale for q/k
ESCALE = SCALE / (Q8 * Q8)  # folded into exp
Q8X = 4.0         # fp8 scale for x (qk-projection path)
Q8W = 64.0        # fp8 scale for Wqk (host-side)
QEVAC = Q8 / (Q8X * Q8W)  # psum holds Q8X*Q8W*qk; evac to Q8*qk
P = 128
DT = 6            # 768 / 128 contraction tiles
ST = 8            # 1024 / 128 sequence tiles
VW = 65           # per-head v width incl. ones column
EXP = mybir.ActivationFunctionType.Exp


def build(ctx, tc: tile.TileContext, aps: dict):
    nc = tc.nc
    hs, wqkv, wqk8, wout, consts, out_d = (
        aps["hs"], aps["wqkv"], aps["wqk8"], aps["wout"], aps["consts"],
        aps["out"])

    pool_p = ctx.enter_context(tc.tile_pool(name="persist", bufs=1))
    pool_s = ctx.enter_context(tc.tile_pool(name="small", bufs=2))
    ps_qv = ctx.enter_context(tc.tile_pool(name="psQV", bufs=2, space="PSUM"))
    ps_sc = ctx.enter_context(tc.tile_pool(name="psSC", bufs=2, space="PSUM"))
    ps_pv = ctx.enter_context(tc.tile_pool(name="psPV", bufs=2, space="PSUM"))

    # ---- persistent SBUF tensors (all fp16 except f32 staging) ----
    x_nat = pool_p.tile([P, 4, D], F16, tag="xnat")        # x rows 0:512
    xT = pool_p.tile([P, DT, S], F16, tag="xT")
    wqk8_sb = pool_p.tile([P, 3, 2, 2 * D], F8, tag="wqk8")
    xT8 = pool_p.tile([P, 3, 2, S], F8, tag="xT8")
    wv_sb = pool_p.tile([P, DT, D], F16, tag="wv")
    wout_sb = pool_p.tile([P, DT, D], F16, tag="wout")
    qkT = pool_p.tile([P, 2 * DT, 2, S], F8, tag="qkT")
    v_buf = pool_p.tile([P, ST, H * VW], F16, tag="vbuf")
    expTs = [pool_p.tile([P, ST, 512], F16, tag=f"expT{i}", name=f"expT{i}")
             for i in range(4)]
    attnT = pool_p.tile([P, DT, S], F16, tag="attnT")
    out_st = pool_p.tile([P, 2, D], F32, tag="outst")
    consts_sb = pool_p.tile([P, 2 * P], F16, tag="consts")
    ident_sb = consts_sb[:, 0:P]
    tri_sb = consts_sb[:, P:2 * P]

    # ---- input DMAs ----
    # SP queue: ident, x rows 0:512 (bootstrap), xbar transposes for
    # cols 512:1024, wout. ACT queue: tri, wqk, wv.
    # Input DMAs: HWDGE setup (~630ns, serialized across both queues) is the
    # load bottleneck, so inputs go as few large multi-dim DMAs, split only
    # where earlier partial arrival feeds the PE sooner. Consumption order:
    # x (transposes) -> wqk q-half (qkT nt0) -> wv (v st0-3) -> wqk k-half
    # (qkT nt6) -> xbar transposes (v st4-7 / qkT cs1) -> wout (tail).
    wv_src = wqkv[0:D, 2 * D:ND3].rearrange("(t p) c -> p t c", p=P)
    wout_src = wout.rearrange("(t p) c -> p t c", p=P)

    # SP queue (shorter dge delay -> carries the critical first x tile).
    # wqk split so pair 0/1 tiles (nt0-1, nt6-7) land first: scores for the
    # first two pairs run before wv even arrives.
    # All input DMAs on the SP queue in consumption order (arrival is
    # transfer-bound, so a single queue costs nothing; ACT's in-order seq
    # must never sit behind DMA issuance or exps stall).
    nc.sync.dma_start(x_nat[:, 0, 0:512], hs[0:P, 0:512])
    nc.sync.dma_start(consts_sb[:], consts)
    nc.sync.dma_start(x_nat[:, 0, 512:D], hs[0:P, 512:D])
    for st in range(1, 4):
        nc.sync.dma_start(x_nat[:, st, :], hs[st * P:(st + 1) * P, :])
    nc.sync.dma_start(wqk8_sb[:, :, :, 0:256], wqk8[:, :, :, 0:256])
    nc.sync.dma_start(wqk8_sb[:, :, :, D:D + 256], wqk8[:, :, :, D:D + 256])
    for half in range(2):
        nc.sync.dma_start(wv_sb[:, :, half * 384:(half + 1) * 384],
                          wv_src[:, :, half * 384:(half + 1) * 384])
    nc.sync.dma_start(wqk8_sb[:, :, :, 256:D], wqk8[:, :, :, 256:D])
    nc.sync.dma_start(wqk8_sb[:, :, :, D + 256:2 * D],
                      wqk8[:, :, :, D + 256:2 * D])
    for dt in range(DT):
        nc.sync.dma_start_transpose(
            xT[:, dt, 512:1024], hs[512:1024, dt * P:(dt + 1) * P])
    nc.sync.dma_start(wout_sb[:], wout_src)

    # ones columns of v_buf (col 64 of each per-head 65-wide slab)
    vb_ones = v_buf.rearrange("p s (h x) -> p s h x", x=VW)[:, :, :, 64]
    nc.vector.memset(vb_ones, 1.0)
    # zero the fp8 DoubleRow pad sub-row of qkT
    nc.vector.memset(qkT[:, :, 1, :].bitcast(F32), 0.0)

    # ---- bootstrap: x rows 0:512 -> xT cols 0:512 via PE transposes ----
    for st in range(4):
        for dp in range(DT // 2):
            dt = 2 * dp
            # alternate psum pools: ps_pv is unused until ~19us, so the
            # boot gets 4 effective buffers instead of 2
            if dp % 2 == 0:
                pt = ps_sc.tile([P, 2, P], F16, tag="psSC")
            else:
                pt = ps_pv.tile([P, 2, P], F16, tag="psPV")
            nc.tensor.transpose(pt[:, 0, :], x_nat[:, st,
                                                   dt * P:(dt + 1) * P],
                                ident_sb[:])
            nc.tensor.transpose(pt[:, 1, :], x_nat[:, st,
                                                   (dt + 1) * P:(dt + 2) * P],
                                ident_sb[:])
            dst = xT[:, dt:dt + 2, st * P:(st + 1) * P]
            nc.vector.tensor_copy(dst, pt[:, :, :])
            d8 = xT8.rearrange("p t i s -> p (t i) s")[
                :, dt:dt + 2, st * P:(st + 1) * P]
            nc.gpsimd.tensor_scalar_mul(d8, dst, Q8X)

    # ---- PE work-block emitters (qkT column-halves, v tiles, out tiles) --
    def emit_qkT(nt, cs, act_evac=False):
        c0, w = (0, 512) if cs == 0 else (512, 512)
        pq = ps_qv.tile([P, 512], F32, tag="psQV")
        for dtp in range(3):
            nc.tensor.matmul(
                pq[:, 0:w],
                wqk8_sb[:, dtp, :, nt * P:(nt + 1) * P],
                xT8[:, dtp, :, c0:c0 + w],
                start=(dtp == 0), stop=(dtp == 2),
                perf_mode=mybir.MatmulPerfMode.DoubleRow)
        # evac re-quantizes to fp8 at scale Q8 (GPSIMD cannot read PSUM)
        if act_evac:
            nc.scalar.mul(qkT[:, nt, 0, c0:c0 + w], pq[:, 0:w], QEVAC)
        else:
            nc.vector.tensor_scalar_mul(qkT[:, nt, 0, c0:c0 + w],
                                        pq[:, 0:w], QEVAC)

    def emit_v(st):
        for vc, (n0, nw) in enumerate(((0, 512), (512, 256))):
            pv = ps_qv.tile([P, 512], F32, tag="psQV")
            for dt in range(DT):
                nc.tensor.matmul(
                    pv[:, 0:nw],
                    xT[:, dt, st * P:(st + 1) * P],
                    wv_sb[:, dt, n0:n0 + nw],
                    start=(dt == 0), stop=(dt == DT - 1))
            dst = v_buf.rearrange("p s (h x) -> p s h x", x=VW)[
                :, st, vc * 8:vc * 8 + nw // HD, 0:HD]
            nc.vector.tensor_copy(
                dst, pv[:, 0:nw].rearrange("p (h x) -> p h x", x=HD))

    def emit_out(st):
        o2 = out_st[:, st % 2]
        for n0, nw in ((0, 512), (512, 256)):
            po = ps_qv.tile([P, 512], F32, tag="psQV")
            for dt in range(DT):
                nc.tensor.matmul(
                    po[:, 0:nw],
                    attnT[:, dt, st * P:(st + 1) * P],
                    wout_sb[:, dt, n0:n0 + nw],
                    start=(dt == 0), stop=(dt == DT - 1))
            # ACT is exp-saturated while st0-5 are woven; DVE idles there.
            # st6 on DVE / st7 on ACT so the two tail evacs run in parallel
            if st == 7:
                nc.scalar.copy(o2[:, n0:n0 + nw], po[:, 0:nw])
            else:
                nc.vector.tensor_copy(o2[:, n0:n0 + nw], po[:, 0:nw])
            if st == 7:
                # tail tile keeps split DMAs so the 512-piece streams out
                # while the 256-piece is still computing
                nc.sync.dma_start(out_d[st * P:(st + 1) * P, n0:n0 + nw],
                                  o2[:, n0:n0 + nw])
        if st < 7:
            # one DMA per row-tile (staging is contiguous): halves the
            # serialized HWDGE setups mid-kernel
            nc.sync.dma_start(out_d[st * P:(st + 1) * P, :], o2[:, :])

    # ---- attention pieces (q-window granular: q0/qw multiples of 128) ----
    def emit_scores(h, q0, qw, eb):
        """scoresT [kp, qp] -> exp -> masked expT for head h, window q0:q0+qw.
        expT buffer eb holds the window at columns 0:qw of k-slot k."""
        r0 = 64 * (h % 2)
        qt, kt = h // 2, 6 + h // 2
        nk = q0 // P + qw // P          # live kp tiles 0 .. nk-1
        for kg in range((nk + 1) // 2):
            k0, k1 = 2 * kg, 2 * kg + 1
            s0 = max(0, k0 * P - q0)
            s1 = max(0, k1 * P - q0) if k1 < nk else qw
            sc = ps_sc.tile([P, 2, 512], F32, tag="psSC")
            # k1 writes from s0 too: the [s0:s1) sliver is dead but exp
            # reads the full [s0:qw] rectangle out of this psum tile
            ks = [(k0, s0)] + ([(k1, s0)] if k1 < nk else [])
            for i, (k, sk) in enumerate(ks):
                nc.tensor.matmul(
                    sc[:, i, sk:qw],
                    qkT[r0:r0 + HD, kt, :, k * P:(k + 1) * P],
                    qkT[r0:r0 + HD, qt, :, q0 + sk:q0 + qw],
                    start=True, stop=True,
                    perf_mode=mybir.MatmulPerfMode.DoubleRow)
            nc.scalar.activation(
                eb[:, k0:k0 + len(ks), s0:qw], sc[:, 0:len(ks), s0:qw], EXP,
                scale=ESCALE)
            for k, sk in ks:
                d = k - q0 // P
                if d >= 0:                  # diagonal block: mask
                    sl = eb[:, k, d * P:(d + 1) * P]
                    # final mask gates the last PV matmul: low-latency DVE;
                    # earlier ones go to the lightly-loaded GPSIMD
                    if k >= nk - 2:
                        nc.vector.tensor_tensor(sl, sl, tri_sb[:],
                                                mybir.AluOpType.mult)
                    else:
                        nc.gpsimd.tensor_tensor(sl, sl, tri_sb[:],
                                                mybir.AluOpType.mult)

    def emit_pv(h, q0, qw, eb):
        r0 = 64 * (h % 2)
        nk = q0 // P + qw // P
        pv = ps_pv.tile([P, 512], F32, tag="psPV")
        for k in range(nk):
            sk = max(0, k * P - q0)
            nc.tensor.matmul(
                pv[0:VW, sk:qw],
                v_buf[:, k, h * VW:(h + 1) * VW],
                eb[:, k, sk:qw],
                start=(k == 0), stop=(k == nk - 1))
        rcp = pool_s.tile([1, 512], F32, tag="dn")
        nc.vector.reciprocal(rcp[:, 0:qw], pv[64:65, 0:qw])
        rep_sb = pool_s.tile([HD, 512], F32, tag="repsb")
        nc.gpsimd.partition_broadcast(rep_sb[:, 0:qw], rcp[:, 0:qw])
        nc.vector.tensor_tensor(
            attnT[r0:r0 + HD, h // 2, q0:q0 + qw],
            pv[0:HD, 0:qw], rep_sb[:, 0:qw], mybir.AluOpType.mult)

    def ebuf(h, c):
        # 4 rotating buffers: up to two pairs' scores/exp in flight at once
        return expTs[h % 4]

    # ---- fused schedule ----
    # Upfront, in DMA-arrival order: qkT for pairs 0-1, their chunk-0
    # scores (PV deferred until wv lands and v st0-3 are computed).
    emit_qkT(0, 0)
    emit_qkT(1, 0)
    emit_qkT(6, 0)
    emit_qkT(7, 0)
    for h in range(4):
        emit_scores(h, 0, 512, ebuf(h, 0))
    for st in range(4):
        emit_v(st)
    for dt in range(DT):
        nc.gpsimd.tensor_scalar_mul(
            xT8[:, dt // 2, dt % 2, 512:1024], xT[:, dt, 512:1024], Q8X)
    emit_pv(0, 0, 512, ebuf(0, 0))
    emit_qkT(2, 0)
    emit_v(4)
    emit_pv(1, 0, 512, ebuf(1, 0))
    emit_qkT(8, 0)
    emit_v(5)
    emit_pv(2, 0, 512, ebuf(2, 0))
    emit_qkT(0, 1)
    emit_qkT(6, 1)
    emit_pv(3, 0, 512, ebuf(3, 0))

    # Software-pipelined sweeps: pair j's chunk-1 step runs after pair
    # (j+2)'s chunk-0 step, spreading the exp-heavy chunk-1 ACT load into
    # the PE-rich chunk-0 region. Two filler slots per step keep the PE fed
    # while exps drain; every filler precedes its first consumer.
    def step(h0, h1, q0, qw, f1=None, f2=None, f3=None):
        emit_scores(h0, q0, qw, ebuf(h0, 0))
        if f1:
            f1()
        emit_scores(h1, q0, qw, ebuf(h1, 0))
        if f2:
            f2()
        emit_pv(h0, q0, qw, ebuf(h0, 0))
        if f3:
            f3()
        emit_pv(h1, q0, qw, ebuf(h1, 0))

    step(4, 5, 0, 512,                  # S20
         lambda: emit_v(6), lambda: emit_v(7),
         lambda: (emit_qkT(1, 1), emit_qkT(7, 1)))
    step(0, 1, 512, 512,                # S01
         lambda: emit_qkT(3, 0), lambda: emit_qkT(9, 0))
    step(6, 7, 0, 512,                  # S30
         lambda: emit_qkT(2, 1), lambda: emit_qkT(8, 1))
    step(2, 3, 512, 512,                # S11
         lambda: emit_qkT(4, 0), lambda: emit_qkT(10, 0))
    step(8, 9, 0, 512,                  # S40
         None, lambda: emit_qkT(3, 1), lambda: emit_qkT(9, 1))
    step(4, 5, 512, 512,                # S21
         lambda: emit_qkT(5, 0), lambda: emit_qkT(11, 0))
    step(10, 11, 0, 512,                # S50
         None, lambda: emit_qkT(4, 1), lambda: emit_qkT(10, 1))
    step(6, 7, 512, 512,                # S31
         lambda: emit_qkT(5, 1), lambda: emit_qkT(11, 1))
    step(8, 9, 512, 512,                # S41
         None, None, lambda: emit_out(0))

    # last pair's chunk 1 in two 256-col halves: out st4/st5 start as soon
    # as the first half is normalized
    emit_scores(10, 512, 256, ebuf(10, 1))
    emit_out(1)
    emit_scores(11, 512, 256, ebuf(11, 1))
    emit_out(2)
    emit_pv(10, 512, 256, ebuf(10, 1))
    emit_out(3)
    emit_pv(11, 512, 256, ebuf(11, 1))
    emit_scores(10, 768, 256, ebuf(10, 1))
    emit_scores(11, 768, 256, ebuf(11, 1))
    emit_out(4)
    emit_pv(10, 768, 256, ebuf(10, 1))
    emit_out(5)
    emit_pv(11, 768, 256, ebuf(11, 1))

    # tail
    emit_out(6)
    emit_out(7)


def build_module():
    nc = bacc.Bacc("TRN2", target_bir_lowering=False, debug=False)
    aps = {
        "hs": nc.dram_tensor("hs", [S, D], F16, kind="ExternalInput").ap(),
        "wqkv": nc.dram_tensor("wqkv", [D, ND3], F16,
                               kind="ExternalInput").ap(),
        "wqk8": nc.dram_tensor("wqk8", [P, 3, 2, 2 * D], F8,
                               kind="ExternalInput").ap(),
        "wout": nc.dram_tensor("wout", [D, D], F16,
                               kind="ExternalInput").ap(),
        "consts": nc.dram_tensor("consts", [P, 2 * P], F16,
                                 kind="ExternalInput").ap(),
        "out": nc.dram_tensor("out", [S, D], F32, kind="ExternalOutput").ap(),
    }
    from contextlib import ExitStack
    with tile.TileContext(nc) as tc, ExitStack() as ctx:
        build(ctx, tc, aps)
    nc.compile()
    return nc


def kernel(hidden_states, Wqkv, bqkv, Wout, bout, _run_kwargs=None):
    hidden_states = np.asarray(hidden_states, dtype=np.float32)
    Wqkv = np.asarray(Wqkv, dtype=np.float32)
    bqkv = np.asarray(bqkv, dtype=np.float32)
    Wout = np.asarray(Wout, dtype=np.float32)
    bout = np.asarray(bout, dtype=np.float32)
    assert not np.any(bqkv), "nonzero qkv bias not supported by this kernel"

    nc = build_module()

    wqkv_h = Wqkv.astype(np.float16)
    wout_h = Wout.astype(np.float16)
    f8 = mybir.dt.np(F8)
    # [d, c] -> [p, dtp, i, c] with d = (2*dtp + i)*128 + p, scaled by Q8W
    wqk8_h = (Wqkv[:, :2 * D] * Q8W).reshape(3, 2, P, 2 * D).transpose(
        2, 0, 1, 3).astype(f8)
    consts = np.concatenate(
        [np.eye(P, dtype=np.float16),
         np.triu(np.ones((P, P), dtype=np.float16))], axis=1)
    in_maps = [
        {
            "hs": hidden_states[b].astype(np.float16),
            "wqkv": wqkv_h,
            "wqk8": wqk8_h,
            "wout": wout_h,
            "consts": consts,
        }
        for b in range(B)
    ]
    res = run_bass_kernel_spmd(nc, in_maps, core_ids=list(range(B)),
                               **(_run_kwargs or {}))
    out = np.stack([res.results[b]["out"] for b in range(B)])
    if np.any(bout):
        out = out + bout
    kernel.last_results = res
    return out.astype(np.float32)
